# revision 32
# baseline (speedup 1.0000x reference)
"""Trainium2 Bass kernel for LocalGraphProjection (perceptual feature pooling).

Pipeline per point: project through 3 cameras, bilinear-sample 3 feature
pyramid levels per view (16/32/64 ch), concat -> [N,112] per view, then
max/mean/std across views -> [N, 3+336] output.

Strategy (v2 — fused pyramid tables):
  - Host folds cameras into one affine per view (fp64 -> fp32 consts).
  - Host rebuilds the 3-level pyramid per view as ONE fused table keyed by
    the L0 cell (i0, j0_pair): entry = [L0 2x4-col patch (128) | L1 2x2
    patch at (i0>>1, j0p) (128) | L2 2x2 patch at (i0>>2, j0p>>1) (256)]
    fp16 = 1024 B. One dma_gather descriptor fetches all 3 levels for one
    (point, view): 3 descriptors/point instead of 9 (the GPSIMD SWDGE
    descriptor rate ~8.4 ns/idx is the kernel bottleneck).
  - L1/L2 bilinear fractions derive from i0/j0 parity bits, so only the
    L0 floor is computed on device.
  - 8 cores data-parallel over points; per-core [128 x 256] point layout;
    per-point math fp32; weighted combine + view stats fp16.
"""

import numpy as np

import concourse.bass as bass
import concourse.bacc as bacc
import concourse.mybir as mybir
from concourse.tile import TileContext
from concourse.bass_utils import run_bass_kernel_spmd
from concourse import library_config

F32 = mybir.dt.float32
F16 = mybir.dt.float16
I16 = mybir.dt.int16
I32 = mybir.dt.int32
ALU = mybir.AluOpType
ACTF = mybir.ActivationFunctionType

PI = float(np.pi)

N_PTS = 262144
N_CORES = 8
N_CORE_PTS = N_PTS // N_CORES  # 32768
P = 128
M = N_CORE_PTS // P  # 256 slots per partition

# points per gather tile
T = 1024
MT = T // P  # 8
NT = M // MT  # 32

TAB_ROWS = 224 * 112  # fused table rows (L0 cell pair-index)
ENT = 512             # fused entry: 128 (L0) + 128 (L1) + 256 (L2) fp16
# per-level channel counts and corner-slot counts within the fused entry
LEV_C = [16, 32, 64]
LEV_S = [8, 4, 4]
LEV_OFF = [0, 128, 256]   # offset of each level's slab in the entry
OUT_OFF = [0, 16, 48]     # channel offset of each level in the 112-ch concat


# ----------------------------------------------------------------- host math
def _camera_affines(cameras: np.ndarray):
    """Per-view affine pc = coord @ A_v + b_v, in float64 (mirrors reference)."""
    cams = cameras.astype(np.float64)

    def cm(param):
        theta = param[0] * (PI / 180.0)
        camy = param[3] * np.sin(param[1] * PI / 180.0)
        lens = param[3] * np.cos(param[1] * PI / 180.0)
        camx = lens * np.cos(theta)
        camz = lens * np.sin(theta)
        Zv = np.array([camx, camy, camz])
        Yv = np.array([camy * np.cos(theta + PI), lens, camy * np.sin(theta + PI)])
        Xv = np.cross(Yv, Zv)
        c = np.stack(
            [Xv / np.linalg.norm(Xv), Yv / np.linalg.norm(Yv), Zv / np.linalg.norm(Zv)]
        )
        return c, Zv

    c0, o0 = cm(cams[0])
    M0 = np.linalg.inv(c0.T)
    A, B = [], []
    for v in range(3):
        cv, ov = cm(cams[v])
        A.append(M0 @ cv.T)            # [3,3]
        B.append((o0 - ov) @ cv.T)     # [3]
    return A, B


def _build_affine_plane(cameras: np.ndarray) -> np.ndarray:
    """[P, 40] fp32: per view v, 12 consts at col v*13:
    [a00,a10,a20,b0, -a01,-a11,-a21,-b1, -a02,-a12,-a22,-b2]."""
    A, B = _camera_affines(cameras)
    row = np.zeros(40, np.float32)
    for v in range(3):
        a, b = A[v], B[v]
        base = v * 13
        row[base + 0 : base + 3] = a[:, 0]
        row[base + 3] = b[0]
        row[base + 4 : base + 7] = -a[:, 1]
        row[base + 7] = -b[1]
        row[base + 8 : base + 11] = -a[:, 2]
        row[base + 11] = -b[2]
    return np.tile(row[None, :], (P, 1))


def _build_tables(img_feat0, img_feat1, img_feat2):
    """Fused per-view tables [TAB_ROWS, 512] fp16.

    Row p = i0*112 + jp covers:
      cols 0:128   L0 patch rows {i0, i0+1c} x cols {2jp..2jp+3 c}, [16c,2r,4col]
      cols 128:256 L1 patch at (i0>>1, jp), [32c, 2r, 2col]
      cols 256:512 L2 patch at (i0>>2, jp>>1), [64c, 2r, 2col]
    (c = clamped to the level's max row/col)
    """
    feats = [np.asarray(img_feat0), np.asarray(img_feat1), np.asarray(img_feat2)]
    i0 = np.arange(TAB_ROWS) // 112
    jp = np.arange(TAB_ROWS) % 112
    map1 = (i0 >> 1) * 112 + jp
    map2 = (i0 >> 2) * 56 + (jp >> 1)
    tabs = []
    for v in range(3):
        # L0: entry (i, jpair) = rows {i, i+1c} x cols {2jp..2jp+3 clamped}
        F = feats[0][v]  # [224,224,16]
        H, W, C = F.shape
        rows = np.stack([np.arange(H), np.minimum(np.arange(H) + 1, H - 1)], 1)
        cols = np.minimum(2 * np.arange(W // 2)[:, None] + np.arange(4)[None, :], W - 1)
        tmp = F[rows]           # [H, 2, W, C]
        tmp = tmp[:, :, cols]   # [H, 2, W/2, 4, C]
        tab0 = np.transpose(tmp, (0, 2, 4, 1, 3)).reshape(H * (W // 2), C * 8)
        tab0 = tab0.astype(np.float16)
        # L1 / L2 base tables: entry (i, j) = 2x2 clamped, [H*W, C*4]
        base = []
        for l in (1, 2):
            F = feats[l][v]
            H, W, C = F.shape
            rows = np.stack([np.arange(H), np.minimum(np.arange(H) + 1, H - 1)], 1)
            cols = np.stack([np.arange(W), np.minimum(np.arange(W) + 1, W - 1)], 1)
            tmp = F[rows]          # [H, 2, W, C]
            tmp = tmp[:, :, cols]  # [H, 2, W, 2, C]
            tab = np.transpose(tmp, (0, 2, 4, 1, 3)).reshape(H * W, C * 4)
            base.append(tab.astype(np.float16))
        fused = np.concatenate([tab0, base[0][map1], base[1][map2]], axis=1)
        tabs.append(np.ascontiguousarray(fused))
    return tabs


# ------------------------------------------------------------- device kernel
def emit_body(nc, tc, pools, dram, m_total=M, mt=MT):
    """Emit the whole per-core program inside an open TileContext."""
    nt = m_total // mt
    sc, wpool, gpool, fpool, opool, ipool = (
        pools["sc"], pools["w"], pools["g"], pools["f"], pools["o"], pools["i"],
    )
    coords_d, afp_d, tabs_d, out_d = (
        dram["coords"], dram["afp"], dram["tabs"], dram["out"],
    )

    V = nc.vector
    G = nc.gpsimd
    S = nc.scalar
    IO = nc.sync

    G.load_library(library_config.mlp)
    nidx_reg = G.alloc_register("nidx")
    G.reg_mov(nidx_reg, P * mt)

    # ---- preload
    coords_sb = sc.tile([P, 3, m_total], F32, tag="coords", name="coords_sb")
    IO.dma_start(out=coords_sb[:, :, :], in_=coords_d[:, :, :].transpose([1, 0, 2]))
    afp = sc.tile([P, 40], F32, tag="afp", name="afp_sb")
    IO.dma_start(out=afp[:, :], in_=afp_d[:, :])

    def ap_s(col):  # [P,1] scalar AP
        return afp[:, col : col + 1]

    cx = coords_sb[:, 0, :]
    cy = coords_sb[:, 1, :]
    cz = coords_sb[:, 2, :]

    # ---- whole-core per-point math (fp32, [P, m_total])
    # per level: view-merged weight tiles [P, 3, m_total, S] fp16
    w_tiles = {
        l: wpool.tile([P, 3, m_total, LEV_S[l]], F16, tag=f"wtl{l}",
                      name=f"wtl{l}")
        for l in range(3)
    }
    wr_tiles = []  # per-view wrapped gather indices
    idx_f = sc.tile([P, 3, m_total], F32, tag="idxf", name="idxf")

    def newt(tag, dt=F32, d3=None, pool=sc):
        shape = [P, m_total] if d3 is None else [P, m_total, d3]
        return pool.tile(shape, dt, tag=tag, name=tag)

    # L0 slots 3 and 7 (unused column of the 2x4 window) are always zero
    V.memset(w_tiles[0][:, :, :, 3], 0)
    V.memset(w_tiles[0][:, :, :, 7], 0)

    for v in range(3):
        base = v * 13
        X = newt("Xs")
        nY = newt("nYs")
        nZ = newt("nZs")
        for out_t, off in ((X, 0), (nY, 4), (nZ, 8)):
            V.tensor_scalar(out_t[:, :], cx, ap_s(base + off + 0),
                            ap_s(base + off + 3), ALU.mult, ALU.add)
            V.scalar_tensor_tensor(
                out_t[:, :], cy, ap_s(base + off + 1), out_t[:, :], ALU.mult, ALU.add
            )
            V.scalar_tensor_tensor(
                out_t[:, :], cz, ap_s(base + off + 2), out_t[:, :], ALU.mult, ALU.add
            )
        rz = newt("rzs")
        V.reciprocal(rz[:, :], nZ[:, :])
        h = newt("hs")
        w_ = newt("ws")
        V.tensor_tensor(h[:, :], nY[:, :], rz[:, :], ALU.mult)
        S.activation(h[:, :], h[:, :], ACTF.Copy, scale=248.0, bias=112.0)
        V.tensor_scalar(h[:, :], h[:, :], 0.0, 223.0, ALU.max, ALU.min)
        V.tensor_tensor(w_[:, :], X[:, :], rz[:, :], ALU.mult)
        S.activation(w_[:, :], w_[:, :], ACTF.Copy, scale=248.0, bias=112.0)
        V.tensor_scalar(w_[:, :], w_[:, :], 0.0, 223.0, ALU.max, ALU.min)

        # floor of h, w (int round-trip + compare fixup; h,w >= 0)
        xi = newt("xis", dt=I32)
        x1 = newt("x1s")
        y1 = newt("y1s")
        fx = newt("fxs")
        fy = newt("fys")
        for (xx, x1x, fxx) in ((h, x1, fx), (w_, y1, fy)):
            V.tensor_copy(xi[:, :], xx[:, :])
            V.tensor_copy(x1x[:, :], xi[:, :])
            V.tensor_tensor(fxx[:, :], x1x[:, :], xx[:, :], ALU.is_gt)
            V.tensor_tensor(x1x[:, :], x1x[:, :], fxx[:, :], ALU.subtract)
            V.tensor_tensor(fxx[:, :], xx[:, :], x1x[:, :], ALU.subtract)

        # parity / mod4 of i0 and j0 (exact small ints in x1/y1)
        pi_ = newt("pis")
        mi4 = newt("mi4s")
        pj_ = newt("pjs")
        mj4 = newt("mj4s")
        for (x1x, pt, mt4) in ((x1, pi_, mi4), (y1, pj_, mj4)):
            V.tensor_copy(xi[:, :], x1x[:, :])
            xi2 = newt("xi2s", dt=I32)
            V.tensor_scalar(xi2[:, :], xi[:, :], 1, None, ALU.bitwise_and)
            V.tensor_copy(pt[:, :], xi2[:, :])
            V.tensor_scalar(xi2[:, :], xi[:, :], 3, None, ALU.bitwise_and)
            V.tensor_copy(mt4[:, :], xi2[:, :])

        # fused index = i0 * 112 + (j0 - pj)/2
        jp2 = newt("jp2s")
        V.tensor_tensor(jp2[:, :], y1[:, :], pj_[:, :], ALU.subtract)
        V.tensor_scalar(jp2[:, :], jp2[:, :], 0.5, None, ALU.mult)
        V.scalar_tensor_tensor(
            idx_f[:, v, :], x1[:, :], 112.0, jp2[:, :], ALU.mult, ALU.add
        )

        # ---- per-view idx relayout for dma_gather (lets view-v gathers
        # start while later views' math still runs).
        # dma_gather wants idx g=(m*128+p) at [g%16, g//16] in a 16-partition
        # block, replicated across the 128 partitions. With p = 32*r4+16*r2+q
        # that is wr[q, t, 8m + 2*r4 + r2] = idx[p, t*mt+m]. Engine APs can
        # only start at partitions 0/32/64/96, so fold in two steps.
        idx_iv = ipool.tile([P, m_total], I16, tag="idxi", name=f"idxi{v}",
                            bufs=1)
        V.tensor_copy(idx_iv[:, :], idx_f[:, v, :])
        i32 = ipool.tile([32, m_total, 4], I16, tag="i32", name=f"i32_{v}",
                         bufs=1)
        i16b = ipool.tile([16, m_total, 4], I16, tag="i16b",
                          name=f"i16b_{v}", bufs=1)
        wrv = ipool.tile([P, 8 * m_total], I16, tag=f"wr{v}", name=f"wr{v}",
                         bufs=1)
        wr_tiles.append(wrv)
        for r4 in range(4):
            V.tensor_copy(i32[:, :, r4], idx_iv[32 * r4 : 32 * (r4 + 1), :])
        IO.dma_start(out=i16b[:, :, :], in_=i32[16:32, :, :])
        wr4 = wrv.rearrange("p (m k r2) -> p m k r2", k=4, r2=2)
        V.tensor_copy(wr4[0:16, :, :, 0], i32[0:16, :, :])
        V.tensor_copy(wr4[0:16, :, :, 1], i16b[:, :, :])
        IO.dma_start(out=wrv[16:32, :], in_=wrv[0:16, :])
        IO.dma_start(out=wrv[32:64, :], in_=wrv[0:32, :])
        IO.dma_start(out=wrv[64:128, :], in_=wrv[0:64, :])

        # ---- L0 weights (8 slots: r*4 + c, col weights with parity handling)
        # row weights (wx1, fx) and col weights (a0, a1, t2) packed into
        # small tiles, then one outer-product op fills slots {0,1,2, 4,5,6};
        # slots 3/7 are zeroed once before the view loop.
        wx1 = newt("wx1s")
        wy1 = newt("wy1s")
        V.tensor_scalar(wx1[:, :], fx[:, :], 0.0, None, ALU.is_gt)
        V.tensor_tensor(wx1[:, :], wx1[:, :], fx[:, :], ALU.subtract)
        V.tensor_scalar(wy1[:, :], fy[:, :], 0.0, None, ALU.is_gt)
        V.tensor_tensor(wy1[:, :], wy1[:, :], fy[:, :], ALU.subtract)
        wt0 = w_tiles[0][:, v]
        rwt = newt("rwts", dt=F32, d3=2)
        cwt = newt("cwts", dt=F32, d3=3)
        t1 = newt("t1s")
        a1 = newt("a1s")
        V.tensor_copy(rwt[:, :, 0], wx1[:, :])
        V.tensor_copy(rwt[:, :, 1], fx[:, :])
        V.tensor_tensor(t1[:, :], wy1[:, :], pj_[:, :], ALU.mult)
        V.tensor_tensor(cwt[:, :, 0], wy1[:, :], t1[:, :], ALU.subtract)
        V.tensor_tensor(cwt[:, :, 2], fy[:, :], pj_[:, :], ALU.mult)
        V.tensor_tensor(a1[:, :], t1[:, :], fy[:, :], ALU.add)
        V.tensor_tensor(cwt[:, :, 1], a1[:, :], cwt[:, :, 2], ALU.subtract)
        wt06 = wt0.rearrange("p m (r c) -> p m r c", r=2)[:, :, :, 0:3]
        rb = rwt.unsqueeze(3).broadcast_to([P, m_total, 2, 3])
        cb = cwt.unsqueeze(2).broadcast_to([P, m_total, 2, 3])
        V.tensor_tensor(wt06, rb, cb, ALU.mult)

        # ---- L1 / L2 weights from parity-derived fractions
        for l, (px, py) in ((1, (pi_, pj_)), (2, (mi4, mj4))):
            inv = 0.5 if l == 1 else 0.25
            f1x = newt("f1xs")
            f1y = newt("f1ys")
            V.tensor_tensor(f1x[:, :], px[:, :], fx[:, :], ALU.add)
            V.tensor_scalar(f1x[:, :], f1x[:, :], inv, None, ALU.mult)
            V.tensor_tensor(f1y[:, :], py[:, :], fy[:, :], ALU.add)
            V.tensor_scalar(f1y[:, :], f1y[:, :], inv, None, ALU.mult)
            u1 = newt("u1s")
            v1 = newt("v1s")
            V.tensor_scalar(u1[:, :], f1x[:, :], 0.0, None, ALU.is_gt)
            V.tensor_tensor(u1[:, :], u1[:, :], f1x[:, :], ALU.subtract)
            V.tensor_scalar(v1[:, :], f1y[:, :], 0.0, None, ALU.is_gt)
            V.tensor_tensor(v1[:, :], v1[:, :], f1y[:, :], ALU.subtract)
            wt = w_tiles[l][:, v]
            V.tensor_tensor(wt[:, :, 0], u1[:, :], v1[:, :], ALU.mult)
            V.tensor_tensor(wt[:, :, 1], u1[:, :], f1y[:, :], ALU.mult)
            V.tensor_tensor(wt[:, :, 2], f1x[:, :], v1[:, :], ALU.mult)
            V.tensor_tensor(wt[:, :, 3], f1x[:, :], f1y[:, :], ALU.mult)

    # ---- per-tile: gather, combine, stats, store
    for t in range(nt):
        sl = slice(t * mt, (t + 1) * mt)
        F_t = fpool.tile([P, mt, 3 * 112], F16, tag="F", name="F", bufs=2)
        for v in range(3):
            q = (t * 3 + v) % 4
            gt = gpool.tile([P, mt, ENT], F16, tag=f"g{v}", name=f"g{v}", bufs=3)
            G.dma_gather(
                gt[:, :, :],
                tabs_d[v][:, :],
                wr_tiles[v][:, t * 8 * mt : (t + 1) * 8 * mt],
                P * mt,
                nidx_reg,
                ENT,
                queue_num=q,
            )
            red = fpool.tile([P, mt, 288], F16, tag="red", name="red", bufs=2)
            for l in range(3):
                Cl, Sl = LEV_C[l], LEV_S[l]
                g4 = gt[:, :, LEV_OFF[l] : LEV_OFF[l] + Cl * Sl].rearrange(
                    "p m (c s) -> p m c s", s=Sl
                )
                wb = (
                    w_tiles[l][:, v, sl, :]
                    .unsqueeze(2)
                    .broadcast_to([P, mt, Cl, Sl])
                )
                V.tensor_tensor(g4, g4, wb, ALU.mult)
            off = v * 112
            # corner-sum as halving adds (2x mode) instead of tensor_reduce
            # (1x): slice innermost stays unit-stride. L1+L2 share S=4 and
            # are adjacent both in the entry and in F, so they fuse.
            u = red[:, :, 0:64].rearrange("p m (c s) -> p m c s", s=4)
            u2 = red[:, :, 64:96].rearrange("p m (c s) -> p m c s", s=2)
            g40 = gt[:, :, 0:128].rearrange("p m (c s) -> p m c s", s=8)
            V.tensor_tensor(u, g40[:, :, :, 0:4], g40[:, :, :, 4:8], ALU.add)
            V.tensor_tensor(u2, u[:, :, :, 0:2], u[:, :, :, 2:4], ALU.add)
            V.tensor_tensor(F_t[:, :, off : off + 16],
                            u2[:, :, :, 0], u2[:, :, :, 1], ALU.add)
            g12 = gt[:, :, 128:512].rearrange("p m (c s) -> p m c s", s=4)
            u12 = red[:, :, 96:288].rearrange("p m (c s) -> p m c s", s=2)
            V.tensor_tensor(u12, g12[:, :, :, 0:2], g12[:, :, :, 2:4], ALU.add)
            V.tensor_tensor(F_t[:, :, off + 16 : off + 112],
                            u12[:, :, :, 0], u12[:, :, :, 1], ALU.add)

        # ---- stats across views
        out_t = opool.tile([P, mt, 336], F16, tag="out", name="out_t", bufs=2)
        F0 = F_t[:, :, 0:112]
        F1 = F_t[:, :, 112:224]
        F2 = F_t[:, :, 224:336]
        fmax = out_t[:, :, 0:112]
        V.tensor_tensor(fmax, F0, F1, ALU.max)
        V.tensor_tensor(fmax, fmax, F2, ALU.max)
        fmean = out_t[:, :, 112:224]
        V.tensor_tensor(fmean, F0, F1, ALU.add)
        V.tensor_tensor(fmean, fmean, F2, ALU.add)
        S.activation(fmean, fmean, ACTF.Copy, scale=1.0 / 3.0)
        # std = sqrt(mean((x - mu)^2)): the centered form keeps small stds
        # accurate in fp16 (x - mu is exact for close values). All three
        # view-subtractions fuse into one op (fmean broadcast over views).
        dt_ = fpool.tile([P, mt, 336], F16, tag="dt_", name="dt_", bufs=2)
        sqa = fpool.tile([P, mt, 336], F16, tag="sqa", name="sqa", bufs=2)
        ssq = fpool.tile([P, mt, 112], F16, tag="ssq", name="ssq", bufs=2)
        d3 = dt_[:, :, :].rearrange("p m (v c) -> p m v c", v=3)
        F3 = F_t[:, :, :].rearrange("p m (v c) -> p m v c", v=3)
        mb = fmean.unsqueeze(2).broadcast_to([P, mt, 3, 112])
        V.tensor_tensor(d3, F3, mb, ALU.subtract)
        S.activation(sqa[:, :, :], dt_[:, :, :], ACTF.Square)
        s3 = sqa[:, :, :].rearrange("p m (v c) -> p m v c", v=3)
        V.tensor_tensor(ssq[:, :, :], s3[:, :, 0], s3[:, :, 1], ALU.add)
        V.tensor_tensor(ssq[:, :, :], ssq[:, :, :], s3[:, :, 2], ALU.add)
        S.activation(out_t[:, :, 224:336], ssq[:, :, :], ACTF.Sqrt, scale=1.0 / 3.0)

        IO.dma_start(out=out_d[:, sl, :], in_=out_t[:, :, :])


def build_kernel(m_total=M, mt=MT):
    """Build the Bass module. Returns nc with dram tensor names."""
    nc = bacc.Bacc("TRN2", num_swdge_queues=4)
    coords = nc.dram_tensor("coords", [3, P, m_total], F32, kind="ExternalInput")
    afp = nc.dram_tensor("afp", [P, 40], F32, kind="ExternalInput")
    tabs = {}
    for v in range(3):
        tabs[v] = nc.dram_tensor(
            f"tab{v}", [TAB_ROWS, ENT], F16, kind="ExternalInput"
        )
    out = nc.dram_tensor("out", [P, m_total, 336], F16, kind="ExternalOutput")

    with nc.allow_low_precision("fp16 sampling kernel"), TileContext(nc) as tc:
        import contextlib

        stack = contextlib.ExitStack()
        pools = {
            "sc": stack.enter_context(tc.tile_pool(name="sc", bufs=1)),
            "w": stack.enter_context(tc.tile_pool(name="w", bufs=1)),
            "g": stack.enter_context(tc.tile_pool(name="g", bufs=3)),
            "f": stack.enter_context(tc.tile_pool(name="f", bufs=1)),
            "o": stack.enter_context(tc.tile_pool(name="o", bufs=1)),
            "i": stack.enter_context(tc.tile_pool(name="i", bufs=2)),
        }
        dram = {
            "coords": coords.ap(),
            "afp": afp.ap(),
            "tabs": {v: t.ap() for v, t in tabs.items()},
            "out": out.ap(),
        }
        with stack:
            emit_body(nc, tc, pools, dram, m_total=m_total, mt=mt)
    nc.compile()
    return nc


# ------------------------------------------------------------------ frontend
_NC_CACHE = {}
TRACE = False
LAST_RES = [None]


def _get_nc():
    if "nc" not in _NC_CACHE:
        _NC_CACHE["nc"] = build_kernel()
    return _NC_CACHE["nc"]


def kernel(coord, img_feat0, img_feat1, img_feat2, cameras):
    coord = np.asarray(coord, np.float32)
    afp = _build_affine_plane(np.asarray(cameras, np.float32))
    tabs = _build_tables(img_feat0, img_feat1, img_feat2)

    nc = _get_nc()
    in_maps = []
    for k in range(N_CORES):
        shard = coord[k * N_CORE_PTS : (k + 1) * N_CORE_PTS]  # [32768, 3]
        cs = np.ascontiguousarray(
            shard.reshape(P, M, 3).transpose(2, 0, 1)
        )  # [3, P, M]
        im = {"coords": cs, "afp": afp}
        for v in range(3):
            im[f"tab{v}"] = tabs[v]
        in_maps.append(im)

    res = run_bass_kernel_spmd(
        nc, in_maps, core_ids=list(range(N_CORES)), trace=TRACE
    )
    LAST_RES[0] = res
    stats = np.concatenate(
        [res.results[k]["out"].reshape(N_CORE_PTS, 336) for k in range(N_CORES)], 0
    ).astype(np.float32)
    return np.concatenate([coord, stats], axis=1)
